# revision 1
# baseline (speedup 1.0000x reference)
"""Trainium2 Bass kernel for nn_DSPLTnet (dual EMA prototype scatter).

Algorithm (per 8-core data-parallel shard, batch dim):
  1. mask = expm((Wpos-Wneg)^2)[:512, -1] via Taylor matvecs (term-1 exact fp32,
     higher terms bf16; A^T materialized via DMA-xbar transpose).
  2. Bucket-stable-sort the shard's 2048 samples by class-range bucket
     (8 buckets x 128 classes), computed on-device with prefix-count matmuls;
     applied via dma_gather row-permutation at HBM load time.
  3. Normalize rows (f and mask*f_aug), fold the per-sample EMA weight
     (1-m)*m^{-cum} into the row scale.  cum (inclusive same-class prefix
     count) comes from triangular one-hot matmuls per bucket.
  4. Per-bucket one-hot matmul scatter -> per-class partial sums S_s[1024,512].
  5. AllGather per-class counts, scale partials by m^{-prior_s(c)}, bf16
     ReduceScatter combines across cores; owner core adds prototypes and
     L2-normalizes its 128-class slice.
Final p = normalize(proto + sum_s m^{-prior} S_s) == reference output (the
reference's global m^count factor is a positive per-class scalar, invariant
under the final L2 normalization).
"""

import math
from contextlib import ExitStack

import numpy as np

import concourse.bass as bass
import concourse.bacc as bacc
import concourse.mybir as mybir
import concourse.tile as tile
from concourse.masks import make_upper_triangular
from concourse import library_config
from concourse.bass import _add_dep_helper

F32 = mybir.dt.float32
BF16 = mybir.dt.bfloat16
I16 = mybir.dt.int16
I32 = mybir.dt.int32

NCORES = 8
B = 16384
BS = B // NCORES            # 2048 samples per core
NCH_IN = BS // 128          # 16 input chunks
D = 512
C = 1000
CPAD = 1024                 # padded classes
NB = 8                      # buckets of 128 classes
CAP = 384                   # slots per bucket (3 chunks)
NSLOT = NB * CAP            # 3072
NCH = NSLOT // 128          # 24 slot chunks
CHB = CAP // 128            # 3 chunks per bucket
M_EMA = 0.99
LAM = -math.log(M_EMA)      # m^-x = exp(LAM*x)
LOG1M = math.log(1.0 - M_EMA)
NTAYLOR = 5                 # Taylor terms w_1..w_5 (err ~1e-7 of mask)
GGRP = 4                    # slot chunks per gather group
NREP = 2                    # idx-table replication (q7 reads 2x16-partition windows for queue 0)
NGRP = NCH // GGRP          # 4 gather calls per tensor


def _ap(t, offset, pattern):
    return bass.AP(tensor=t, offset=offset, ap=[list(p) for p in pattern])


def build_kernel(debug=False):
    nc = bacc.Bacc(None, target_bir_lowering=False, debug=False,
                   num_devices=NCORES)

    f_sh = nc.dram_tensor("f_sh", [BS, D], F32, kind="ExternalInput")
    fa_sh = nc.dram_tensor("fa_sh", [BS, D], F32, kind="ExternalInput")
    y_sh = nc.dram_tensor("y_sh", [BS], I32, kind="ExternalInput")
    proto2 = nc.dram_tensor("proto2", [2, 128, D], F32, kind="ExternalInput")
    wpos = nc.dram_tensor("wpos", [513, 513], F32, kind="ExternalInput")
    wneg = nc.dram_tensor("wneg", [513, 513], F32, kind="ExternalInput")
    rmask = nc.dram_tensor("rmask", [1, NCORES], F32, kind="ExternalInput")

    pout2 = nc.dram_tensor("pout2", [2, 128, D], F32, kind="ExternalOutput")

    # internal DRAM scratch
    dstg = nc.dram_tensor("dstg", [BS], I16)
    table = nc.dram_tensor("tbl", [NSLOT, 128], I16)
    a_dram = nc.dram_tensor("a_dram", [640, 640], BF16)
    mcol_dram = nc.dram_tensor("mcol_dram", [640], F32)
    agin = nc.dram_tensor("agin", [CPAD], F32)
    agout = nc.dram_tensor("agout", [NCORES, CPAD], F32)
    rsin = nc.dram_tensor("rsin", [2 * CPAD, D], BF16)
    rsout = nc.dram_tensor("rsout", [256, D], BF16)

    dbg = {}
    if debug:
        dbg["dst"] = nc.dram_tensor("dbg_dst", [128, NCH_IN], F32, kind="ExternalOutput")
        dbg["yhat"] = nc.dram_tensor("dbg_yhat", [128, NCH], F32, kind="ExternalOutput")
        dbg["cum"] = nc.dram_tensor("dbg_cum", [128, NCH], F32, kind="ExternalOutput")
        dbg["mask"] = nc.dram_tensor("dbg_mask", [1, D], F32, kind="ExternalOutput")
        dbg["cnts"] = nc.dram_tensor("dbg_cnts", [128, NB], F32, kind="ExternalOutput")
        dbg["sp"] = nc.dram_tensor("dbg_sp", [CPAD, D], F32, kind="ExternalOutput")

    with tile.TileContext(nc) as tc, ExitStack() as ctx:
        _body(ctx, tc, locals())
    return nc


def _body(ctx, tc, t):
    nc = tc.nc
    f_sh, fa_sh, y_sh = t["f_sh"], t["fa_sh"], t["y_sh"]
    proto2 = t["proto2"]
    wpos, wneg, rmask = t["wpos"], t["wneg"], t["rmask"]
    pout2 = t["pout2"]
    dstg, table, a_dram, mcol_dram = t["dstg"], t["table"], t["a_dram"], t["mcol_dram"]
    agin, agout = t["agin"], t["agout"]
    rsin, rsout = t["rsin"], t["rsout"]
    dbg = t["dbg"]

    singles = ctx.enter_context(tc.tile_pool(name="singles", bufs=1))
    small = ctx.enter_context(tc.tile_pool(name="small", bufs=4))
    big = ctx.enter_context(tc.tile_pool(name="big", bufs=2))
    pipe = ctx.enter_context(tc.tile_pool(name="pipe", bufs=2))
    pfam = ctx.enter_context(tc.tile_pool(name="pfam", bufs=2))
    pcnt = ctx.enter_context(tc.tile_pool(name="pcnt", bufs=1, space="PSUM"))
    pmm = ctx.enter_context(tc.tile_pool(name="pmm", bufs=3, space="PSUM"))
    psc = ctx.enter_context(tc.tile_pool(name="psc", bufs=2, space="PSUM"))

    # ---------------- constants ----------------
    iota_insts = []
    iota_row_i = singles.tile([128, 128], I32)
    iota_insts.append(nc.gpsimd.iota(iota_row_i, pattern=[[1, 128]], channel_multiplier=0))
    iota_row = singles.tile([128, 128], F32)
    nc.vector.tensor_copy(iota_row, iota_row_i)

    thr9_i = singles.tile([128, 9], I32)
    iota_insts.append(nc.gpsimd.iota(thr9_i, pattern=[[128, 9]], channel_multiplier=0))
    thr9 = singles.tile([128, 9], F32)
    nc.vector.tensor_copy(thr9, thr9_i)

    ut128 = singles.tile([128, 128], BF16)
    make_upper_triangular(nc, ut128[:, :], val=1.0, diag=True)
    ones128 = singles.tile([128, 128], BF16)
    nc.vector.memset(ones128, 1.0)
    onescol = singles.tile([128, 1], BF16)
    nc.vector.memset(onescol, 1.0)

    zcol = singles.tile([128, 1], F32)
    nc.vector.memset(zcol, 0.0)
    lncol = singles.tile([128, 1], F32)
    nc.vector.memset(lncol, LOG1M)

    probej = singles.tile([1, 16], F32)

    def probe(ap_1elem):
        # tiny DVE read so the DVE vector clock observes a DMA completion;
        # keeps later DVE ops at <=1 inline sync wait (HW TT limit)
        nc.vector.tensor_copy(out=probej[0:1, 0:1], in_=ap_1elem)

    rmask_bc = singles.tile([128, NCORES], F32)
    nc.sync.dma_start(out=rmask_bc, in_=_ap(rmask, 0, [[0, 128], [1, NCORES]]))
    probe(rmask_bc[0:1, 0:1])

    prc = singles.tile([128, 2, D], F32)
    nc.sync.dma_start(out=prc, in_=_ap(proto2, 0, [[D, 128], [128 * D, 2], [1, D]]))
    probe(prc[0:1, 0, 0:1])

    # ---------------- load y ----------------
    # sample i -> [i % 128, i // 128]
    y_all_i = singles.tile([128, NCH_IN], I32)
    nc.sync.dma_start(out=y_all_i, in_=_ap(y_sh, 0, [[1, 128], [128, NCH_IN]]))
    y_all = singles.tile([128, NCH_IN], F32)
    nc.vector.tensor_copy(out=y_all, in_=y_all_i)

    # ---------------- bucket sort: dst slot per sample ----------------
    o8b = singles.tile([128, NCH_IN, NB], BF16)
    o8f = singles.tile([128, NCH_IN, NB], F32)
    ge9a = singles.tile([128, NCH_IN, 9], F32)
    dstall = singles.tile([128, NCH_IN], I16)
    dst_f32 = singles.tile([128, NCH_IN], F32)
    bsum_all = singles.tile([128, NCH_IN], F32)

    # all chunks at once: ge9a[p,t,b] = [thr_b <= y[p,t]]
    y_exp = bass.AP(tensor=y_all.tensor, offset=y_all.offset,
                    ap=[y_all.ap[0], y_all.ap[1], [0, 9]])
    thr_exp = bass.AP(tensor=thr9.tensor, offset=thr9.offset,
                      ap=[thr9.ap[0], [0, NCH_IN], thr9.ap[1]])
    nc.vector.tensor_tensor(out=ge9a, in0=thr_exp, in1=y_exp,
                            op=mybir.AluOpType.is_le)
    nc.vector.tensor_reduce(out=bsum_all, in_=ge9a, axis=mybir.AxisListType.X,
                            op=mybir.AluOpType.add)
    nc.vector.tensor_tensor(out=o8f, in0=ge9a[:, :, 0:NB], in1=ge9a[:, :, 1:NB + 1],
                            op=mybir.AluOpType.subtract)
    nc.vector.tensor_copy(out=o8b, in_=o8f)

    # running prefix sums of o8 over chunks (for the 2-MM R8 form)
    o8run = singles.tile([128, NCH_IN, NB], BF16)
    run8 = small.tile([128, NB], F32, tag="run8")
    for tt in range(NCH_IN):
        if tt == 0:
            nc.vector.tensor_copy(out=run8, in_=o8f[:, 0, :])
        else:
            nc.vector.tensor_tensor(out=run8, in0=run8, in1=o8f[:, tt, :],
                                    op=mybir.AluOpType.add)
        nc.vector.tensor_copy(out=o8run[:, tt, :], in_=run8)

    r8all = pmm.tile([128, NCH_IN, NB], F32, tag="mmscr")
    for tt in range(NCH_IN):
        if tt > 0:
            nc.tensor.matmul(out=r8all[:, tt, :], lhsT=ones128,
                             rhs=o8run[:, tt - 1, :], start=True, stop=False)
        nc.tensor.matmul(out=r8all[:, tt, :], lhsT=ut128, rhs=o8b[:, tt, :],
                         start=(tt == 0), stop=True)
    rank_all = singles.tile([128, NCH_IN], F32)
    for tt in range(NCH_IN):
        junk8 = small.tile([128, NB], F32, tag="junk8")
        nc.vector.scalar_tensor_tensor(
            out=junk8, in0=r8all[:, tt, :], scalar=1.0, in1=o8f[:, tt, :],
            op0=mybir.AluOpType.mult, op1=mybir.AluOpType.mult,
            accum_out=rank_all[:, tt : tt + 1],
        )
    nc.vector.scalar_tensor_tensor(
        out=dst_f32, in0=bsum_all, scalar=float(CAP), in1=rank_all,
        op0=mybir.AluOpType.mult, op1=mybir.AluOpType.add,
    )
    nc.vector.tensor_scalar(
        out=dstall, in0=dst_f32, scalar1=float(CAP + 1), scalar2=None,
        op0=mybir.AluOpType.subtract,
    )
    if dbg:
        nc.sync.dma_start(out=dbg["dst"][:, :], in_=dst_f32)

    # ---------------- staging roundtrip + payload scatter ----------------
    nc.sync.dma_start(out=_ap(dstg, 0, [[1, 128], [128, NCH_IN]]), in_=dstall)
    idxs_dst = singles.tile([128, 128], I16)
    nc.gpsimd.memset(idxs_dst, 0)
    for r in range(NREP):
        eng = nc.sync if r % 2 == 0 else nc.scalar
        eng.dma_start(out=idxs_dst[16 * r : 16 * r + 16, :],
                      in_=_ap(dstg, 0, [[1, 16], [16, 128]]))

    payload = singles.tile([128, NCH_IN, 128], I16)
    nc.gpsimd.memset(payload, 0)
    iota_insts.append(nc.gpsimd.iota(payload[:, :, 0], pattern=[[128, NCH_IN]], channel_multiplier=1))
    nc.vector.tensor_scalar(
        out=payload[:, :, 1], in0=y_all, scalar1=1.0, scalar2=None,
        op0=mybir.AluOpType.add,
    )

    ztbl = singles.tile([128, NSLOT], I16)
    nc.vector.memset(ztbl, 0)
    nc.sync.dma_start(out=_ap(table, 0, [[NSLOT, 128], [1, NSLOT]]), in_=ztbl)

    sc_inst = nc.gpsimd.dma_scatter_add(
        out_ap=_ap(table, 0, [[128, NSLOT], [1, 128]]),
        in_ap=payload[:, :, :],
        idxs_ap=idxs_dst[:, :],
        num_idxs=BS, num_idxs_reg=BS, elem_size=128,
    )


    # ---------------- readback: gather idxs + permuted y ----------------
    idxs_src = singles.tile([128, NSLOT // 16], I16)
    nc.gpsimd.memset(idxs_src, 0)
    W0 = 8 * GGRP  # first gather group's idx window
    for r in range(NREP):
        eng = nc.sync if r % 2 == 0 else nc.scalar
        eng.dma_start(out=idxs_src[16 * r : 16 * r + 16, 0:W0],
                      in_=_ap(table, 0, [[128, 16], [2048, W0]]))
    for r in range(NREP):
        eng = nc.sync if r % 2 == 0 else nc.scalar
        eng.dma_start(out=idxs_src[16 * r : 16 * r + 16, W0:],
                      in_=_ap(table, W0 * 16 * 128, [[128, 16], [2048, NSLOT // 16 - W0]]))

    yh_i16 = singles.tile([128, NCH], I16)
    nc.scalar.dma_start(out=yh_i16, in_=_ap(table, 1, [[128, 128], [16384, NCH]]))
    yh = singles.tile([128, NCH], F32)  # y+1 in slot order; 0 = pad
    nc.vector.tensor_copy(out=yh, in_=yh_i16)
    if dbg:
        nc.sync.dma_start(out=dbg["yhat"][:, :], in_=yh)

    # ---------------- mask chain (overlaps with the above) ----------------
    wp = singles.tile([128, 5, 513], F32)
    wn = singles.tile([128, 5, 513], F32)
    nc.sync.dma_start(out=wp[:, 0:4, :], in_=_ap(wpos, 0, [[513, 128], [513 * 128, 4], [1, 513]]))
    nc.sync.dma_start(out=wn[:, 0:4, :], in_=_ap(wneg, 0, [[513, 128], [513 * 128, 4], [1, 513]]))
    nc.sync.dma_start(out=wp[0:1, 4, :], in_=wpos[512:513, :])
    nc.sync.dma_start(out=wn[0:1, 4, :], in_=wneg[512:513, :])
    probe(wp[0:1, 0, 0:1])
    probe(wn[0:1, 0, 0:1])
    probe(wp[0:1, 4, 0:1])
    probe(wn[0:1, 4, 0:1])

    abf = singles.tile([128, 5, 640], BF16)
    nc.gpsimd.memset(abf, 0)
    wdiff = big.tile([128, 513], F32, tag="wdiff")
    for ci in range(5):
        rows = 128 if ci < 4 else 1
        wdiff = big.tile([128, 513], F32, tag="wdiff")
        nc.vector.tensor_tensor(out=wdiff[:rows, :], in0=wp[:rows, ci, :],
                                in1=wn[:rows, ci, :], op=mybir.AluOpType.subtract)
        nc.vector.tensor_tensor(out=abf[:rows, ci, 0:513], in0=wdiff[:rows, :],
                                in1=wdiff[:rows, :], op=mybir.AluOpType.mult)
    ident = singles.tile([128, 128], BF16)
    from concourse.masks import make_identity
    make_identity(nc, ident[:, :])
    at = singles.tile([128, 5, 640], BF16)  # A^T, row chunk cj = cols of A
    for cj in range(5):
        for ci in range(5):
            tp = pmm.tile([128, 256], BF16, tag="mmscr")
            tp = tp[:, :128]
            nc.tensor.transpose(out=tp, in_=abf[:, ci, 128 * cj : 128 * cj + 128],
                                identity=ident[:, :])
            nc.scalar.activation(out=at[:, cj, 128 * ci : 128 * ci + 128], in_=tp,
                                 func=mybir.ActivationFunctionType.Copy)

    # w1 = A[:, 512] exact fp32 = (wp - wn)[:, 512]^2
    wcol = singles.tile([128, 5], F32)
    wcol_bf = singles.tile([128, 5], BF16)
    mask_col = singles.tile([128, 5], F32)
    ccol = small.tile([128, 5], F32, tag="ccol")
    nc.gpsimd.memset(ccol, 0.0)
    for ci in range(5):
        rows = 128 if ci < 4 else 1
        nc.vector.tensor_tensor(out=ccol[:rows, ci : ci + 1],
                                in0=wp[:rows, ci, 512:513], in1=wn[:rows, ci, 512:513],
                                op=mybir.AluOpType.subtract)
    nc.vector.tensor_tensor(out=wcol, in0=ccol, in1=ccol, op=mybir.AluOpType.mult)
    nc.vector.tensor_copy(out=mask_col, in_=wcol)
    nc.vector.tensor_copy(out=wcol_bf, in_=wcol)

    for k in range(2, NTAYLOR + 1):
        mvfull = pmm.tile([128, 128], F32, tag="mmscr")
        mv = mvfull[:, :5]
        for mi in range(5):
            for ki in range(5):
                nc.tensor.matmul(
                    out=mv[:, mi : mi + 1],
                    lhsT=at[:, ki, 128 * mi : 128 * mi + 128],
                    rhs=wcol_bf[:, ki : ki + 1],
                    start=(ki == 0), stop=(ki == 4),
                )
        wcol = singles.tile([128, 5], F32, tag=f"wcol{k}")
        nc.vector.tensor_scalar(out=wcol, in0=mv, scalar1=1.0 / k, scalar2=None,
                                op0=mybir.AluOpType.mult)
        wcol_bf = singles.tile([128, 5], BF16, tag=f"wcolb{k}")
        nc.vector.tensor_copy(out=wcol_bf, in_=wcol)
        nc.vector.tensor_tensor(out=mask_col, in0=mask_col, in1=wcol,
                                op=mybir.AluOpType.add)

    nc.sync.dma_start(out=_ap(mcol_dram, 0, [[1, 128], [128, 5]]), in_=mask_col)
    mask_bc = singles.tile([128, D], F32)
    nc.sync.dma_start(out=mask_bc, in_=_ap(mcol_dram, 0, [[0, 128], [1, D]]))
    probe(mask_bc[0:1, 0:1])
    if dbg:
        nc.sync.dma_start(out=dbg["mask"][:, :], in_=mask_bc[0:1, :])

    # ---------------- one-hot, counts, cum, weights (slot order) ----------------
    obf = singles.tile([128, NCH, 128], BF16)
    cnt_ps = pcnt.tile([128, NB], F32, tag="cnt")
    gscale_p = singles.tile([128, NCH], F32)
    gscale_py = singles.tile([128, NCH], F32)
    wexp_all = singles.tile([128, NCH], F32)

    # one-hot for all 24 chunks in two batched ops (bf16 for MMs, f32 for extract)
    bias24_i = singles.tile([128, NB, CHB], I32)
    iota_insts.append(nc.gpsimd.iota(bias24_i, pattern=[[128, NB], [0, CHB]],
                                     base=1, channel_multiplier=0))
    bias24 = singles.tile([128, NCH], F32)
    nc.vector.tensor_copy(out=bias24, in_=bias24_i)
    yloc24 = singles.tile([128, NCH], F32)
    nc.vector.tensor_tensor(out=yloc24, in0=yh, in1=bias24,
                            op=mybir.AluOpType.subtract)
    iota_exp = bass.AP(tensor=iota_row.tensor, offset=iota_row.offset,
                       ap=[iota_row.ap[0], [0, NCH], iota_row.ap[1]])
    yloc_exp = bass.AP(tensor=yloc24.tensor, offset=yloc24.offset,
                       ap=[yloc24.ap[0], yloc24.ap[1], [0, 128]])
    nc.vector.scalar_tensor_tensor(
        out=obf, in0=iota_exp, scalar=1.0, in1=yloc_exp,
        op0=mybir.AluOpType.mult, op1=mybir.AluOpType.is_equal)

    for b in range(NB):
        for j in range(CHB):
            c = CHB * b + j
            nc.tensor.matmul(out=cnt_ps[:, b : b + 1], lhsT=obf[:, c, :],
                             rhs=onescol, start=(j == 0), stop=(j == CHB - 1))
        rps3 = pmm.tile([128, CHB, 128], F32, tag="mmscr")
        for j in range(CHB):
            c = CHB * b + j
            for cp in range(CHB * b, c):
                nc.tensor.matmul(out=rps3[:, j, :], lhsT=ones128,
                                 rhs=obf[:, cp, :], start=(cp == CHB * b), stop=False)
            nc.tensor.matmul(out=rps3[:, j, :], lhsT=ut128, rhs=obf[:, c, :],
                             start=(j == 0), stop=True)
        for j in range(CHB):
            c = CHB * b + j
            junk = small.tile([128, 128], F32, tag="junkr")
            nc.vector.scalar_tensor_tensor(
                out=junk, in0=rps3[:, j, :], scalar=1.0, in1=obf[:, c, :],
                op0=mybir.AluOpType.mult, op1=mybir.AluOpType.mult,
                accum_out=wexp_all[:, c : c + 1],
            )

    # w = (1-m) * m^-cum = exp(LAM*cum + log(1-m)), all chunks in one op
    nc.scalar.activation(out=wexp_all, in_=wexp_all,
                         func=mybir.ActivationFunctionType.Exp,
                         bias=lncol[:, :], scale=LAM)

    if dbg:
        nc.sync.dma_start(out=dbg["cum"][:, :], in_=wexp_all)

    cnts = singles.tile([128, NB], F32)
    nc.vector.tensor_copy(out=cnts, in_=cnt_ps)
    if dbg:
        nc.sync.dma_start(out=dbg["cnts"][:, :], in_=cnts)
    nc.sync.dma_start(out=_ap(agin, 0, [[1, 128], [128, NB]]), in_=cnts)
    nc.gpsimd.collective_compute(
        "AllGather", mybir.AluOpType.bypass,
        replica_groups=[list(range(NCORES))],
        ins=[agin.ap().opt()], outs=[agout.ap().opt()],
    )
    agt = singles.tile([128, NCORES, NB], F32)
    nc.sync.dma_start(out=agt,
                      in_=_ap(agout, 0, [[1, 128], [CPAD, NCORES], [128, NB]]))
    prior = singles.tile([128, NB], F32)
    nc.vector.memset(prior, 0.0)
    for sprime in range(NCORES):
        nc.vector.scalar_tensor_tensor(
            out=prior, in0=agt[:, sprime, :],
            scalar=rmask_bc[:, sprime : sprime + 1], in1=prior,
            op0=mybir.AluOpType.mult, op1=mybir.AluOpType.add,
        )
    scalefac = singles.tile([128, NB], F32)
    nc.scalar.activation(out=scalefac, in_=prior,
                         func=mybir.ActivationFunctionType.Exp,
                         bias=zcol[:, :], scale=LAM)

    # ---------------- gathers + normalize + G ----------------
    gp = singles.tile([128, NCH, D], BF16)
    gpy = singles.tile([128, NCH, D], BF16)

    for g in range(NGRP):
        c0 = g * GGRP
        fg = big.tile([128, GGRP, D], F32, tag="fg")
        fag = big.tile([128, GGRP, D], F32, tag="fag")
        g1 = nc.gpsimd.dma_gather(
            out_ap=fg[:, :, :], in_ap=f_sh.ap(),
            idxs_ap=idxs_src[:, 8 * c0 : 8 * (c0 + GGRP)],
            num_idxs=GGRP * 128, num_idxs_reg=GGRP * 128, elem_size=D,
        )
        g2 = nc.gpsimd.dma_gather(
            out_ap=fag[:, :, :], in_ap=fa_sh.ap(),
            idxs_ap=idxs_src[:, 8 * c0 : 8 * (c0 + GGRP)],
            num_idxs=GGRP * 128, num_idxs_reg=GGRP * 128, elem_size=D,
        )
        ss = small.tile([128, 2 * GGRP], F32, tag="ss")
        fam_g = pfam.tile([128, GGRP, D], BF16, tag="fam")
        mask_exp = bass.AP(tensor=mask_bc.tensor, offset=mask_bc.offset,
                           ap=[mask_bc.ap[0], [0, GGRP], mask_bc.ap[1]])
        nc.vector.tensor_tensor(out=fam_g, in0=fag[:, :, :], in1=mask_exp,
                                op=mybir.AluOpType.mult)
        for cc in range(GGRP):
            sq = pipe.tile([128, D], BF16, tag="sqscr")
            nc.scalar.activation(out=sq, in_=fg[:, cc, :],
                                 func=mybir.ActivationFunctionType.Square,
                                 bias=zcol[:, :], accum_out=ss[:, 2 * cc : 2 * cc + 1])
            sq2 = pipe.tile([128, D], BF16, tag="sqscr2")
            nc.scalar.activation(out=sq2, in_=fam_g[:, cc, :],
                                 func=mybir.ActivationFunctionType.Square,
                                 bias=zcol[:, :], accum_out=ss[:, 2 * cc + 1 : 2 * cc + 2])
        nrm = small.tile([128, 2 * GGRP], F32, tag="nrm")
        nc.scalar.activation(out=nrm, in_=ss,
                             func=mybir.ActivationFunctionType.Sqrt,
                             bias=zcol[:, :])
        nc.vector.tensor_scalar(out=nrm, in0=nrm, scalar1=1e-12, scalar2=None,
                                op0=mybir.AluOpType.max)
        nc.vector.reciprocal(out=nrm, in_=nrm)
        nrm_p = bass.AP(tensor=nrm.tensor, offset=nrm.offset,
                        ap=[nrm.ap[0], [2, GGRP]])
        nrm_py = bass.AP(tensor=nrm.tensor, offset=nrm.offset + 1,
                         ap=[nrm.ap[0], [2, GGRP]])
        nc.vector.tensor_tensor(out=gscale_p[:, c0 : c0 + GGRP], in0=nrm_p,
                                in1=wexp_all[:, c0 : c0 + GGRP],
                                op=mybir.AluOpType.mult)
        nc.vector.tensor_tensor(out=gscale_py[:, c0 : c0 + GGRP], in0=nrm_py,
                                in1=wexp_all[:, c0 : c0 + GGRP],
                                op=mybir.AluOpType.mult)
        gsp_exp = bass.AP(tensor=gscale_p.tensor, offset=gscale_p.offset + c0,
                          ap=[gscale_p.ap[0], [1, GGRP], [0, D]])
        gspy_exp = bass.AP(tensor=gscale_py.tensor, offset=gscale_py.offset + c0,
                           ap=[gscale_py.ap[0], [1, GGRP], [0, D]])
        nc.vector.scalar_tensor_tensor(
            out=gp[:, c0 : c0 + GGRP, :], in0=fg[:, :, :], scalar=1.0,
            in1=gsp_exp, op0=mybir.AluOpType.mult, op1=mybir.AluOpType.mult)
        nc.vector.scalar_tensor_tensor(
            out=gpy[:, c0 : c0 + GGRP, :], in0=fam_g, scalar=1.0,
            in1=gspy_exp, op0=mybir.AluOpType.mult, op1=mybir.AluOpType.mult)

    # ---------------- scatter matmuls + scaled evac + RS ----------------
    sall = singles.tile([128, 2 * NB, D], BF16)
    for b in range(NB):
        sp = psc.tile([128, D], F32, tag="sp")
        spy = psc.tile([128, D], F32, tag="spy")
        for j in range(CHB):
            c = CHB * b + j
            nc.tensor.matmul(out=sp, lhsT=obf[:, c, :], rhs=gp[:, c, :],
                             start=(j == 0), stop=(j == CHB - 1))
            nc.tensor.matmul(out=spy, lhsT=obf[:, c, :], rhs=gpy[:, c, :],
                             start=(j == 0), stop=(j == CHB - 1))
        nc.scalar.activation(out=sall[:, 2 * b, :], in_=sp,
                             func=mybir.ActivationFunctionType.Copy,
                             scale=scalefac[:, b : b + 1])
        nc.vector.tensor_scalar(out=sall[:, 2 * b + 1, :], in0=spy,
                                scalar1=scalefac[:, b : b + 1], scalar2=None,
                                op0=mybir.AluOpType.mult)
        if dbg:
            spdbg = big.tile([128, D], F32, tag="spdbg")
            nc.vector.tensor_scalar(out=spdbg, in0=sp,
                                    scalar1=scalefac[:, b : b + 1], scalar2=None,
                                    op0=mybir.AluOpType.mult)
            nc.sync.dma_start(out=dbg["sp"][128 * b : 128 * (b + 1), :], in_=spdbg)
    nc.sync.dma_start(out=_ap(rsin, 0, [[D, 128], [128 * D, NB], [1, D]]),
                      in_=sall[:, 0:NB, :])
    nc.scalar.dma_start(out=_ap(rsin, NB * 128 * D, [[D, 128], [128 * D, NB], [1, D]]),
                      in_=sall[:, NB:2 * NB, :])
    nc.gpsimd.collective_compute(
        "ReduceScatter", mybir.AluOpType.add,
        replica_groups=[list(range(NCORES))],
        ins=[rsin.ap().opt()], outs=[rsout.ap().opt()],
    )

    # ---------------- finalize: + proto, L2 normalize (p & py together) ----
    s_b2 = big.tile([128, 2, D], BF16, tag="sbfin")
    nc.sync.dma_start(out=s_b2, in_=_ap(rsout, 0, [[D, 128], [128 * D, 2], [1, D]]))
    nc.vector.scalar_tensor_tensor(
        out=prc, in0=s_b2, scalar=1.0, in1=prc,
        op0=mybir.AluOpType.mult, op1=mybir.AluOpType.add)
    sqf = big.tile([128, D], BF16, tag="sqfin")
    ssf = small.tile([128, 2], F32, tag="ssfin")
    nc.scalar.activation(out=sqf, in_=prc[:, 0, :],
                         func=mybir.ActivationFunctionType.Square,
                         bias=zcol[:, :], accum_out=ssf[:, 0:1])
    sqf2 = big.tile([128, D], BF16, tag="sqfin2")
    nc.scalar.activation(out=sqf2, in_=prc[:, 1, :],
                         func=mybir.ActivationFunctionType.Square,
                         bias=zcol[:, :], accum_out=ssf[:, 1:2])
    nc.scalar.activation(out=ssf, in_=ssf,
                         func=mybir.ActivationFunctionType.Sqrt,
                         bias=zcol[:, :])
    nc.vector.tensor_scalar(out=ssf, in0=ssf, scalar1=1e-12, scalar2=None,
                            op0=mybir.AluOpType.max)
    nc.vector.reciprocal(out=ssf, in_=ssf)
    pfin = big.tile([128, 2, D], F32, tag="pfin")
    ssf_exp = bass.AP(tensor=ssf.tensor, offset=ssf.offset,
                      ap=[ssf.ap[0], ssf.ap[1], [0, D]])
    nc.vector.scalar_tensor_tensor(
        out=pfin, in0=prc, scalar=1.0, in1=ssf_exp,
        op0=mybir.AluOpType.mult, op1=mybir.AluOpType.mult)
    nc.sync.dma_start(out=_ap(pout2, 0, [[D, 128], [128 * D, 2], [1, D]]), in_=pfin)


def make_in_maps(f, f_aug, y, prototypes, prototypes_y, weight_pos, weight_neg):
    f = np.ascontiguousarray(np.asarray(f, dtype=np.float32))
    f_aug = np.ascontiguousarray(np.asarray(f_aug, dtype=np.float32))
    y = np.ascontiguousarray(np.asarray(y).astype(np.int32))
    prototypes = np.asarray(prototypes, dtype=np.float32)
    prototypes_y = np.asarray(prototypes_y, dtype=np.float32)
    wp = np.ascontiguousarray(np.asarray(weight_pos, dtype=np.float32))
    wn = np.ascontiguousarray(np.asarray(weight_neg, dtype=np.float32))

    ppad = np.zeros((CPAD, D), np.float32)
    ppad[:C] = prototypes
    pypad = np.zeros((CPAD, D), np.float32)
    pypad[:C] = prototypes_y
    proto2 = np.stack([ppad.reshape(NCORES, 128, D),
                       pypad.reshape(NCORES, 128, D)], axis=1)

    in_maps = []
    for s in range(NCORES):
        rm = np.zeros((1, NCORES), np.float32)
        rm[0, :s] = 1.0
        in_maps.append({
            "f_sh": np.ascontiguousarray(f[s * BS:(s + 1) * BS]),
            "fa_sh": np.ascontiguousarray(f_aug[s * BS:(s + 1) * BS]),
            "y_sh": np.ascontiguousarray(y[s * BS:(s + 1) * BS]),
            "proto2": np.ascontiguousarray(proto2[s]),
            "wpos": wp,
            "wneg": wn,
            "rmask": rm,
        })
    return in_maps


_NC_CACHE = {}


def run_kernel(in_maps, trace=False):
    from concourse.bass_utils import run_bass_kernel_spmd

    if "nc" not in _NC_CACHE:
        nc = build_kernel(debug=False)
        if not nc.is_finalized():
            nc.finalize()
        _NC_CACHE["nc"] = nc
    nc = _NC_CACHE["nc"]
    return run_bass_kernel_spmd(nc, in_maps, core_ids=list(range(NCORES)),
                                trace=trace)


def kernel(f, f_aug, y, prototypes, prototypes_y, weight_pos, weight_neg):
    in_maps = make_in_maps(f, f_aug, y, prototypes, prototypes_y,
                           weight_pos, weight_neg)
    res = run_kernel(in_maps).results
    p = np.concatenate([res[s]["pout2"][0] for s in range(NCORES)], axis=0)[:C]
    py = np.concatenate([res[s]["pout2"][1] for s in range(NCORES)], axis=0)[:C]
    return p.astype(np.float32), py.astype(np.float32)

